# Initial kernel scaffold
#
"""Trainium2 Bass kernel for nn_HardSigmoidRT.

Computes out = where(z < e2, e0, where(z <= e3, e0 + (e1-e0)/(e3-e2)*(z-e2), e1))
where eta=[e0,e1,e2,e3] comes from a tiny per-sample MLP on [N,4] inputs.

Strategy:
  - The eta MLP is O(N*4*64) flops -> computed on host in float64 numpy.
  - The piecewise-linear map over z [128,1024,512] f32 (256 MiB in/out) is the
    real work: pure data parallelism over the sample axis N across 8 cores.
  - Per core: 16 samples, each sample = 1024*512 = 128*4096 f32, processed as
    one [128, 4096] SBUF tile (2 MiB DMAs).
  - Device math per tile (per-partition scalar operands from a small params
    tile):  t = (z - e2) * slope   (DVE tensor_scalar, 2 ops)
            u = min(max(t, 0), e1-e0)  (DVE tensor_scalar, 2 ops, in-place)
            out = u + e0           (ACT Identity with per-partition bias)
    This matches the reference's float32 op ordering exactly in the left
    plateau and interior; right-plateau deviations are <= ~2 ulp of e1.
"""

import numpy as np

N = 128
H, W = 1024, 512
NCORES = 8
NPER = N // NCORES            # 16 samples per core
P = 128                       # SBUF partitions
SAMPLE = H * W                # 524288 = P * 4096
FREE = SAMPLE // P            # 4096
ROWS = NPER * P               # 2048 rows per core

_cache = {}


def _eta_host(rt_, noise, X_min, X_max, Y_min, Y_max, W1, b1, W2, b2):
    """float64 mirror of the reference _eta; returns float32 [N,4]."""
    rt = rt_.astype(np.float64)
    sig = 1.0 / (1.0 + np.exp(-rt))
    RTn = np.concatenate([sig, np.zeros(1)])
    Xmin = X_min.astype(np.float64)
    Xmax = X_max.astype(np.float64)
    RT = RTn * (Xmax - Xmin) + Xmin
    RT_noisy = RT[None, :] * noise.astype(np.float64)
    ext = np.stack(
        [RT_noisy[:, 0], RT_noisy[:, 1], RT_noisy[:, 2],
         RT_noisy[:, 1] / RT_noisy[:, 2]], axis=1)
    xn = (ext - Xmin) / (Xmax - Xmin)
    h = np.maximum(xn @ W1.astype(np.float64) + b1.astype(np.float64), 0.0)
    logits = h @ W2.astype(np.float64) + b2.astype(np.float64)
    eta_n = 1.0 / (1.0 + np.exp(-logits))
    eta = eta_n * (Y_max.astype(np.float64) - Y_min.astype(np.float64)) \
        + Y_min.astype(np.float64)
    return eta.astype(np.float32)


def _build_module():
    import concourse.bass as bass
    import concourse.mybir as mybir
    from concourse.tile import TileContext

    f32 = mybir.dt.float32
    Alu = mybir.AluOpType
    Act = mybir.ActivationFunctionType

    nc = bass.Bass(trn_type="TRN2", target_bir_lowering=False, debug=False,
                   num_devices=NCORES)
    z_in = nc.dram_tensor("z", [ROWS, FREE], f32, kind="ExternalInput")
    par_in = nc.dram_tensor("params", [P, 4 * NPER], f32, kind="ExternalInput")
    out = nc.dram_tensor("out", [ROWS, FREE], f32, kind="ExternalOutput")

    with TileContext(nc) as tc:
        with tc.tile_pool(name="const", bufs=1) as cpool, \
             tc.tile_pool(name="io", bufs=3) as pool:
            par = cpool.tile([P, 4 * NPER], f32)
            nc.sync.dma_start(out=par[:], in_=par_in[:])
            for s in range(NPER):
                e2 = par[:, 4 * s + 0:4 * s + 1]
                sl = par[:, 4 * s + 1:4 * s + 2]
                d = par[:, 4 * s + 2:4 * s + 3]
                e0 = par[:, 4 * s + 3:4 * s + 4]
                zt = pool.tile([P, FREE], f32, tag="zt")
                nc.sync.dma_start(out=zt[:], in_=z_in[s * P:(s + 1) * P, :])
                # t = (z - e2) * slope
                nc.vector.tensor_scalar(zt[:], zt[:], e2, sl,
                                        Alu.subtract, Alu.mult)
                # u = min(max(t, 0), e1-e0)
                nc.vector.tensor_scalar(zt[:], zt[:], 0.0, d,
                                        Alu.max, Alu.min)
                ot = pool.tile([P, FREE], f32, tag="ot")
                # out = u + e0
                nc.scalar.activation(ot[:], zt[:], Act.Identity,
                                     bias=e0, scale=1.0)
                # store from the ACT hwdge ring (separate FIFO from loads)
                nc.scalar.dma_start(out=out[s * P:(s + 1) * P, :], in_=ot[:])
    return nc


def _get_module():
    if "nc" not in _cache:
        _cache["nc"] = _build_module()
    return _cache["nc"]


def kernel(**inputs):
    from concourse.bass_utils import run_bass_kernel_spmd

    z = np.ascontiguousarray(inputs["z"], dtype=np.float32)
    eta = _eta_host(inputs["rt_"], inputs["noise"], inputs["X_min"],
                    inputs["X_max"], inputs["Y_min"], inputs["Y_max"],
                    inputs["W1"], inputs["b1"], inputs["W2"], inputs["b2"])
    e0, e1, e2, e3 = eta[:, 0], eta[:, 1], eta[:, 2], eta[:, 3]
    # f32 ops, same order as reference: slope = (e1-e0)/(e3-e2)
    d = e1 - e0
    slope = d / (e3 - e2)
    # per-sample param quad (e2, slope, d, e0), replicated over partitions
    quad = np.stack([e2, slope, d, e0], axis=1)          # [N, 4] f32

    nc = _get_module()
    in_maps = []
    for c in range(NCORES):
        zc = z[c * NPER:(c + 1) * NPER].reshape(ROWS, FREE)
        qc = quad[c * NPER:(c + 1) * NPER].reshape(1, 4 * NPER)
        pc = np.ascontiguousarray(np.broadcast_to(qc, (P, 4 * NPER)),
                                  dtype=np.float32)
        in_maps.append({"z": zc, "params": pc})

    res = run_bass_kernel_spmd(nc, in_maps, core_ids=list(range(NCORES)))
    outs = [r["out"].reshape(NPER, H, W) for r in res.results]
    return np.concatenate(outs, axis=0)


# revision 4
# speedup vs baseline: 1.0099x; 1.0099x over previous
"""Trainium2 Bass kernel for nn_HardSigmoidRT.

Computes out = where(z < e2, e0, where(z <= e3, e0 + (e1-e0)/(e3-e2)*(z-e2), e1))
where eta=[e0,e1,e2,e3] comes from a tiny per-sample MLP on [N,4] inputs.

Strategy:
  - The eta MLP is O(N*4*64) flops -> computed on host in float64 numpy.
  - The piecewise-linear map over z [128,1024,512] f32 (256 MiB in/out) is the
    real work: pure data parallelism over the sample axis N across 8 cores.
  - Per core: 16 samples, each sample = 1024*512 = 128*4096 f32, processed as
    one [128, 4096] SBUF tile (2 MiB DMAs).
  - Device math per tile (per-partition scalar operands from a small params
    tile):  t = (z - e2) * slope   (DVE tensor_scalar, 2 ops)
            u = min(max(t, 0), e1-e0)  (DVE tensor_scalar, 2 ops, in-place)
            out = u + e0           (ACT Identity with per-partition bias)
    This matches the reference's float32 op ordering exactly in the left
    plateau and interior; right-plateau deviations are <= ~2 ulp of e1.
"""

import numpy as np

N = 128
H, W = 1024, 512
NCORES = 8
NPER = N // NCORES            # 16 samples per core
P = 128                       # SBUF partitions
SAMPLE = H * W                # 524288 = P * 4096
FREE = SAMPLE // P            # 4096
ROWS = NPER * P               # 2048 rows per core

_cache = {}


def _eta_host(rt_, noise, X_min, X_max, Y_min, Y_max, W1, b1, W2, b2):
    """float64 mirror of the reference _eta; returns float32 [N,4]."""
    rt = rt_.astype(np.float64)
    sig = 1.0 / (1.0 + np.exp(-rt))
    RTn = np.concatenate([sig, np.zeros(1)])
    Xmin = X_min.astype(np.float64)
    Xmax = X_max.astype(np.float64)
    RT = RTn * (Xmax - Xmin) + Xmin
    RT_noisy = RT[None, :] * noise.astype(np.float64)
    ext = np.stack(
        [RT_noisy[:, 0], RT_noisy[:, 1], RT_noisy[:, 2],
         RT_noisy[:, 1] / RT_noisy[:, 2]], axis=1)
    xn = (ext - Xmin) / (Xmax - Xmin)
    h = np.maximum(xn @ W1.astype(np.float64) + b1.astype(np.float64), 0.0)
    logits = h @ W2.astype(np.float64) + b2.astype(np.float64)
    eta_n = 1.0 / (1.0 + np.exp(-logits))
    eta = eta_n * (Y_max.astype(np.float64) - Y_min.astype(np.float64)) \
        + Y_min.astype(np.float64)
    return eta.astype(np.float32)


def _build_module(reps=1):
    import concourse.bacc as bacc
    import concourse.mybir as mybir
    from concourse.tile import TileContext

    f32 = mybir.dt.float32
    Alu = mybir.AluOpType
    Act = mybir.ActivationFunctionType

    nc = bacc.Bacc(trn_type="TRN2", target_bir_lowering=False, debug=False,
                   num_devices=NCORES)
    z_in = nc.dram_tensor("z", [ROWS, FREE], f32, kind="ExternalInput")
    par_in = nc.dram_tensor("params", [P, 4 * NPER], f32, kind="ExternalInput")
    out = nc.dram_tensor("out", [ROWS, FREE], f32, kind="ExternalOutput")

    with TileContext(nc) as tc:
        with tc.tile_pool(name="const", bufs=1) as cpool, \
             tc.tile_pool(name="io", bufs=3) as pool:
            par = cpool.tile([P, 4 * NPER], f32)
            nc.sync.dma_start(out=par[:], in_=par_in[:])
            for _ in range(reps):
                for s in range(NPER):
                    e2 = par[:, 4 * s + 0:4 * s + 1]
                    sl = par[:, 4 * s + 1:4 * s + 2]
                    d = par[:, 4 * s + 2:4 * s + 3]
                    e0 = par[:, 4 * s + 3:4 * s + 4]
                    zt = pool.tile([P, FREE], f32, tag="zt")
                    nc.sync.dma_start(out=zt[:], in_=z_in[s * P:(s + 1) * P, :])
                    # t = (z - e2) * slope
                    nc.vector.tensor_scalar(zt[:], zt[:], e2, sl,
                                            Alu.subtract, Alu.mult)
                    # u = min(max(t, 0), e1-e0)
                    nc.vector.tensor_scalar(zt[:], zt[:], 0.0, d,
                                            Alu.max, Alu.min)
                    ot = pool.tile([P, FREE], f32, tag="ot")
                    # out = u + e0
                    nc.scalar.activation(ot[:], zt[:], Act.Identity,
                                         bias=e0, scale=1.0)
                    # store from the ACT hwdge ring (separate FIFO from loads)
                    nc.scalar.dma_start(out=out[s * P:(s + 1) * P, :],
                                        in_=ot[:])
    nc.compile()
    return nc


def _get_module():
    if "nc" not in _cache:
        _cache["nc"] = _build_module()
    return _cache["nc"]


def kernel(**inputs):
    from concourse.bass_utils import run_bass_kernel_spmd

    z = np.ascontiguousarray(inputs["z"], dtype=np.float32)
    eta = _eta_host(inputs["rt_"], inputs["noise"], inputs["X_min"],
                    inputs["X_max"], inputs["Y_min"], inputs["Y_max"],
                    inputs["W1"], inputs["b1"], inputs["W2"], inputs["b2"])
    e0, e1, e2, e3 = eta[:, 0], eta[:, 1], eta[:, 2], eta[:, 3]
    # f32 ops, same order as reference: slope = (e1-e0)/(e3-e2)
    d = e1 - e0
    slope = d / (e3 - e2)
    # per-sample param quad (e2, slope, d, e0), replicated over partitions
    quad = np.stack([e2, slope, d, e0], axis=1)          # [N, 4] f32

    nc = _get_module()
    in_maps = []
    for c in range(NCORES):
        zc = z[c * NPER:(c + 1) * NPER].reshape(ROWS, FREE)
        qc = quad[c * NPER:(c + 1) * NPER].reshape(1, 4 * NPER)
        pc = np.ascontiguousarray(np.broadcast_to(qc, (P, 4 * NPER)),
                                  dtype=np.float32)
        in_maps.append({"z": zc, "params": pc})

    res = run_bass_kernel_spmd(nc, in_maps, core_ids=list(range(NCORES)))
    outs = [r["out"].reshape(NPER, H, W) for r in res.results]
    return np.concatenate(outs, axis=0)
